# revision 11
# baseline (speedup 1.0000x reference)
"""nn_LESA Trainium2 kernel: 8-core SPMD Bass/Tile implementation.

Sharding: attention-group parallel (core g owns head-group g for all 4
batches: qkv projection slice, relative-position attention, grouped-conv
group, and the 64-channel output slice). Two small collectives knit the
full-channel 1x1-conv chain together: an AllGather of the grouped-conv
output and a per-batch AllReduce of the W_r partial products.

The relative-position einsums (qr/kr/sve) are computed as dense GEMMs
against `relative` in (i, t=i-j+HW-1) coordinates, then mapped back to
(i, j) with diagonal SBUF->SBUF DMA access patterns (per-partition
shifted windows); kr additionally needs PE transposes, as does attn for
the sv/sve contractions.
"""
import os
import sys
import types

import numpy as np
import ml_dtypes

import concourse.bass as bass
from concourse import mybir
from concourse.tile import TileContext

G = 8
NB = 4          # batches
QK = 32
VP = 64
HW = 784
T = 112         # pixel tile
NT = 7          # HW // T
WW = 896        # padded shear window width (8*T)
BR = 512
EPS = 1e-5
SHIFT = 20.0    # exp(x - SHIFT); cancels in normalization

f32 = mybir.dt.float32
bf16 = mybir.dt.bfloat16
BF = ml_dtypes.bfloat16

_DEBUG = bool(int(os.environ.get("BASSK_DEBUG", "0")))


def _split_multiwaits(nc):
    """walrus in this env allows 1 sync-wait per instruction; hoist extras
    onto same-engine NOPs placed just before the waiting instruction."""
    for f in nc.m.functions:
        for b in f.blocks:
            out = []
            for inst in b.instructions:
                si = inst.sync_info
                if si is not None and len(si.on_wait) > 1:
                    waits = list(si.on_wait)
                    for k, w in enumerate(waits[:-1]):
                        nop = mybir.InstNoOp(name=f"{inst.name}_w{k}", ins=[], outs=[])
                        nop.engine = inst.engine
                        nop.sync_info = mybir.SyncInfo(on_wait=[w], on_update=[])
                        out.append(nop)
                    inst.sync_info = mybir.SyncInfo(
                        on_wait=waits[-1:], on_update=list(si.on_update))
                out.append(inst)
            b.instructions = out


def _install_profile_hook():
    """antenv.axon_hooks is absent in this image; synthesize it so
    run_bass_kernel_spmd(trace=True) can reach the ctypes NTFF hook."""
    if "antenv.axon_hooks" in sys.modules:
        return
    try:
        from trn_agent_boot.trn_boot import _ntff_profile_via_ctypes
        hook = _ntff_profile_via_ctypes("/opt/axon/libaxon_pjrt.so")
    except Exception:
        hook = None
    mod = types.ModuleType("antenv.axon_hooks")
    mod.get_axon_ntff_profile_hook = lambda: hook
    mod.set_axon_ntff_profile_hook = lambda h: None
    sys.modules["antenv.axon_hooks"] = mod


def _diag(t_ap, extra_off, pstep, count, fstep, fcount):
    return bass.AP(t_ap.tensor, t_ap.offset + extra_off,
                   [[pstep, count], [fstep, fcount]])


def _build():
    nc = bass.Bass(num_devices=G)
    dt_in = {}

    def inp(name, shape, dt):
        dt_in[name] = nc.dram_tensor(name, list(shape), dt, kind="ExternalInput")
        return dt_in[name]

    xf = inp("xf", (NB, BR, HW), f32)
    xg = inp("xg", (NB, VP, HW), bf16)
    wqkvT = inp("wqkvT", (4, 128, 128), f32)
    bqkv = inp("bqkv", (128, 1), f32)
    relq = inp("relq", (QK, 2 * HW - 1), f32)
    relk = inp("relk", (QK, 2 * HW - 1), f32)
    relvT = inp("relvT", (2 * HW, VP), bf16)      # row 2*HW-1 zero-padded
    w3T = inp("w3T", (9, VP, VP), bf16)
    wx1T = inp("wx1T", (4, 128, VP), bf16)
    bx = inp("bx", (VP, 1), f32)
    wruT = inp("wruT", (VP, BR), bf16)
    wrbT = inp("wrbT", (VP, BR), bf16)
    br = inp("br", (128, 4), f32)
    wpT = inp("wpT", (4, 128, VP), bf16)
    bp = inp("bp", (VP, 1), f32)
    bsum = inp("bsum", (VP, 1), f32)
    identb = inp("identb", (128, 128), bf16)
    identf = inp("identf", (128, 128), f32)

    out_d = nc.dram_tensor("out", [NB, VP, HW], f32, kind="ExternalOutput")
    dbg = {}
    if _DEBUG:
        dbg["qkv0"] = nc.dram_tensor("dbg_qkv0", [128, HW], f32, kind="ExternalOutput")
        dbg["sim0"] = nc.dram_tensor("dbg_sim0", [T, HW], f32, kind="ExternalOutput")
        dbg["attn0"] = nc.dram_tensor("dbg_attn0", [T, WW], f32, kind="ExternalOutput")
        dbg["ash0"] = nc.dram_tensor("dbg_ash0", [T, WW], f32, kind="ExternalOutput")
        dbg["bin0"] = nc.dram_tensor("dbg_bin0", [VP, HW], f32, kind="ExternalOutput")
        dbg["u30"] = nc.dram_tensor("dbg_u30", [VP, HW], f32, kind="ExternalOutput")
        dbg["un0"] = nc.dram_tensor("dbg_un0", [VP, HW], f32, kind="ExternalOutput")
        dbg["gate0"] = nc.dram_tensor("dbg_gate0", [VP, HW], f32, kind="ExternalOutput")
        dbg["sve0"] = nc.dram_tensor("dbg_sve0", [VP, HW], f32, kind="ExternalOutput")
        dbg["sv0"] = nc.dram_tensor("dbg_sv0", [VP, HW], f32, kind="ExternalOutput")

    AF = mybir.ActivationFunctionType
    OP = mybir.AluOpType

    from contextlib import ExitStack
    with TileContext(nc) as tc, ExitStack() as stk:
        cst = stk.enter_context(tc.tile_pool(name="cst", bufs=1))
        wrk = stk.enter_context(tc.tile_pool(name="wrk", bufs=2))
        drp = stk.enter_context(tc.tile_pool(name="drp", bufs=1, space="DRAM"))
        # PSUM: sim pair (2 banks) + small pool (6 banks) = 8
        ps_sim = stk.enter_context(tc.tile_pool(name="ps_sim", bufs=1, space="PSUM"))
        ps_sm = stk.enter_context(tc.tile_pool(name="ps_sm", bufs=6, space="PSUM"))

        def smt(p, q, dt, name):
            return ps_sm.tile([p, q], dt, tag="sm", name=name)

        # ---------------- constants into SBUF ----------------
        xpad = cst.tile([VP, NB * 900], bf16, name="xpad")
        nc.gpsimd.memset(xpad[:], 0.0)
        for b in range(NB):
            dst = bass.AP(xpad[:].tensor, xpad[:].offset + b * 900 + 31,
                          [[NB * 900, VP], [30, 28], [1, 28]])
            nc.sync.dma_start(dst, xg[b])
        wqkvT_sb = cst.tile([128, 512], f32, name="wqkvT_sb")
        nc.sync.dma_start(
            wqkvT_sb[:].rearrange("p (k f) -> p k f", k=4),
            bass.AP(wqkvT, 0, [[128, 128], [128 * 128, 4], [1, 128]]))
        bqkv_sb = cst.tile([128, 1], f32, name="bqkv_sb")
        nc.sync.dma_start(bqkv_sb[:], bqkv[:])
        relq_sb = cst.tile([QK, 2 * HW - 1], f32, name="relq_sb")
        nc.sync.dma_start(relq_sb[:], relq[:])
        relk_sb = cst.tile([QK, 2 * HW - 1], f32, name="relk_sb")
        nc.sync.dma_start(relk_sb[:], relk[:])
        relvT_sb = cst.tile([T, 14 * VP], bf16, name="relvT_sb")
        nc.sync.dma_start(
            relvT_sb[:].rearrange("p (k f) -> p k f", k=14),
            bass.AP(relvT, 0, [[VP, T], [T * VP, 14], [1, VP]]))
        w3T_sb = cst.tile([VP, 9 * VP], bf16, name="w3T_sb")
        nc.sync.dma_start(
            w3T_sb[:].rearrange("p (k f) -> p k f", k=9),
            bass.AP(w3T, 0, [[VP, VP], [VP * VP, 9], [1, VP]]))
        wx1T_sb = cst.tile([128, 4 * VP], bf16, name="wx1T_sb")
        nc.sync.dma_start(
            wx1T_sb[:].rearrange("p (k f) -> p k f", k=4),
            bass.AP(wx1T, 0, [[VP, 128], [128 * VP, 4], [1, VP]]))
        wruT_sb = cst.tile([VP, BR], bf16, name="wruT_sb")
        nc.sync.dma_start(wruT_sb[:], wruT[:])
        wrbT_sb = cst.tile([VP, BR], bf16, name="wrbT_sb")
        nc.sync.dma_start(wrbT_sb[:], wrbT[:])
        wpT_sb = cst.tile([128, 4 * VP], bf16, name="wpT_sb")
        nc.sync.dma_start(
            wpT_sb[:].rearrange("p (k f) -> p k f", k=4),
            bass.AP(wpT, 0, [[VP, 128], [128 * VP, 4], [1, VP]]))
        bx_sb = cst.tile([VP, 1], f32, name="bx_sb")
        nc.sync.dma_start(bx_sb[:], bx[:])
        br_sb = cst.tile([128, 4], f32, name="br_sb")
        nc.sync.dma_start(br_sb[:], br[:])
        bp_sb = cst.tile([VP, 1], f32, name="bp_sb")
        nc.sync.dma_start(bp_sb[:], bp[:])
        bsum_sb = cst.tile([VP, 1], f32, name="bsum_sb")
        nc.sync.dma_start(bsum_sb[:], bsum[:])
        idb_sb = cst.tile([128, 128], bf16, name="idb_sb")
        nc.sync.dma_start(idb_sb[:], identb[:])
        idf_sb = cst.tile([128, 128], f32, name="idf_sb")
        nc.sync.dma_start(idf_sb[:], identf[:])
        zero_sb = cst.tile([128, 1], f32, name="zero_sb")
        nc.gpsimd.memset(zero_sb[:], 0.0)
        nshift_sb = cst.tile([128, 1], f32, name="nshift_sb")
        nc.gpsimd.memset(nshift_sb[:], -SHIFT)

        # persistent per-batch tiles
        q_sb = [cst.tile([QK, HW], f32, name=f"q{b}") for b in range(NB)]
        k_sb = [cst.tile([QK, HW], f32, name=f"k{b}") for b in range(NB)]
        v_sb = [cst.tile([VP, HW], bf16, name=f"v{b}") for b in range(NB)]
        vT_b = [cst.tile([T, NT * VP], bf16, name=f"vT{b}") for b in range(NB)]
        unary_b = [cst.tile([VP, HW], f32, name=f"un{b}") for b in range(NB)]
        ru_b = [cst.tile([VP, HW], bf16, name=f"ru{b}") for b in range(NB)]
        bin_b = [cst.tile([VP, HW], f32, name=f"bin{b}") for b in range(NB)]
        rb_b = [cst.tile([VP, HW], bf16, name=f"rb{b}") for b in range(NB)]
        attnT_sb = [cst.tile([T, HW], bf16, name=f"attnT{j}") for j in range(NT)]
        ash_f = [cst.tile([T, HW], bf16, name=f"ashf{j}") for j in range(NT)]

        # collective buffers (DRAM pool tiles so Tile tracks deps)
        cc1_in = drp.tile([NB * VP, HW], bf16, name="cc1_in")
        cc1_out = drp.tile([G * NB * VP, HW], bf16, name="cc1_out",
                           addr_space="Shared")
        cc2_in = [drp.tile([BR, HW], bf16, name=f"cc2i{b}") for b in range(NB)]
        cc2_out = [drp.tile([BR, HW], bf16, name=f"cc2o{b}",
                            addr_space="Shared") for b in range(NB)]

        NCH = (448, 336)  # HW split, bank-aligned psum chunks

        # ---------------- qkv projection + conv3x3 (all batches) ------------
        for b in range(NB):
            p0 = smt(128, 448, f32, f"qv0_{b}")
            p1 = smt(128, 336, f32, f"qv1_{b}")
            for kt in range(4):
                rhs = wrk.tile([128, HW], f32, name="xft", tag="xft", bufs=2)
                nc.sync.dma_start(
                    rhs[:], bass.AP(xf, b * BR * HW + kt * 128 * HW,
                                    [[HW, 128], [1, HW]]))
                lhsT = wqkvT_sb[:, kt * 128:(kt + 1) * 128]
                nc.tensor.matmul(p0[:], lhsT, rhs[:, 0:448],
                                 start=(kt == 0), stop=(kt == 3))
                nc.tensor.matmul(p1[:], lhsT, rhs[:, 448:HW],
                                 start=(kt == 0), stop=(kt == 3))
            qkv_f = wrk.tile([128, HW], f32, name="qkv_f", tag="qkv_f")
            nc.scalar.activation(qkv_f[:, 0:448], p0[:], AF.Identity,
                                 bias=bqkv_sb[:, 0:1])
            nc.scalar.activation(qkv_f[:, 448:HW], p1[:], AF.Identity,
                                 bias=bqkv_sb[:, 0:1])
            # partition-rebase q/k/v to base 0 (matmul needs equal bases)
            nc.sync.dma_start(q_sb[b][:], qkv_f[0:QK, :])
            nc.sync.dma_start(k_sb[b][:], qkv_f[QK:2 * QK, :])
            nc.gpsimd.dma_start(v_sb[b][:], qkv_f[2 * QK:128, :])
            # v reversed copy then plain transposes -> vT (descending j chunks)
            vrev = wrk.tile([VP, HW], bf16, name="vrev", tag="vrev")
            rev_out = bass.AP(vrev[:].tensor, vrev[:].offset + HW - 1,
                              [[HW, VP], [-1, HW]])
            nc.scalar.activation(rev_out, v_sb[b][:], AF.Copy)
            for j in range(NT):
                pt = smt(T, VP, bf16, f"vtp_{b}_{j}")
                nc.tensor.transpose(pt[:], vrev[:, j * T:(j + 1) * T],
                                    idb_sb[0:VP, 0:VP])
                nc.any.tensor_copy(vT_b[b][:, j * VP:(j + 1) * VP], pt[:])
            # grouped 3x3 conv (unary branch pre-1x1)
            c0 = smt(VP, 392, f32, f"cv0_{b}")
            c1 = smt(VP, 392, f32, f"cv1_{b}")
            for k in range(9):
                dy, dx = divmod(k, 3)
                lhsT = w3T_sb[:, k * VP:(k + 1) * VP]
                for h, cp in ((0, c0), (1, c1)):
                    rhs = bass.AP(xpad[:].tensor,
                                  xpad[:].offset + b * 900 + dy * 30 + dx + h * 420,
                                  [[NB * 900, VP], [30, 14], [1, 28]])
                    nc.tensor.matmul(cp[:], lhsT, rhs,
                                     start=(k == 0), stop=(k == 8))
            u3 = wrk.tile([VP, HW], bf16, name="u3", tag="u3")
            nc.scalar.activation(u3[:, 0:392], c0[:], AF.Copy)
            nc.scalar.activation(u3[:, 392:HW], c1[:], AF.Copy)
            if _DEBUG and b == 0:
                nc.gpsimd.dma_start(dbg["u30"][:], u3[:])
                pass
            nc.sync.dma_start(cc1_in[b * VP:(b + 1) * VP, :], u3[:])

        nc.gpsimd.collective_compute(
            "AllGather", OP.bypass, replica_groups=[list(range(G))],
            ins=[cc1_in[:]], outs=[cc1_out[:]])

        # ---------------- attention per batch ----------------
        for b in range(NB):
            # kr precompute: a_t_rev GEMM windows + shear -> A[j, i] rows
            for J in range(NT):
                w0 = 672 - J * T
                a0 = smt(T, 448, f32, f"at0_{b}_{J}")
                a1 = smt(T, 447, f32, f"at1_{b}_{J}")
                lhsT = k_sb[b][:, J * T:(J + 1) * T]
                nc.tensor.matmul(a0[:], lhsT, relk_sb[:, w0:w0 + 448],
                                 start=True, stop=True)
                nc.tensor.matmul(a1[:], lhsT, relk_sb[:, w0 + 448:w0 + 895],
                                 start=True, stop=True)
                araw = wrk.tile([T, WW], bf16, name="araw", tag="araw")
                nc.scalar.activation(araw[:, 0:448], a0[:], AF.Copy)
                nc.scalar.activation(araw[:, 448:895], a1[:], AF.Copy)
                nc.sync.dma_start(
                    ash_f[J][:], _diag(araw[:], 111, WW - 1, T, 1, HW))

            sv0 = smt(VP, 448, f32, f"sv0_{b}")
            sv1 = smt(VP, 336, f32, f"sv1_{b}")
            sve_sb = wrk.tile([VP, HW], f32, name="sve_sb", tag="svesb")

            for I in range(NT):
                w0 = 672 - I * T
                # qr GEMM + shear
                q0 = smt(T, 448, f32, f"qr0_{b}_{I}")
                q1 = smt(T, 447, f32, f"qr1_{b}_{I}")
                lq = q_sb[b][:, I * T:(I + 1) * T]
                nc.tensor.matmul(q0[:], lq, relq_sb[:, w0:w0 + 448],
                                 start=True, stop=True)
                nc.tensor.matmul(q1[:], lq, relq_sb[:, w0 + 448:w0 + 895],
                                 start=True, stop=True)
                qraw = wrk.tile([T, WW], bf16, name="qraw", tag="qraw")
                nc.scalar.activation(qraw[:, 0:448], q0[:], AF.Copy)
                nc.scalar.activation(qraw[:, 448:895], q1[:], AF.Copy)
                qsh = wrk.tile([T, HW], bf16, name="qsh", tag="qsh")
                nc.sync.dma_start(qsh[:], _diag(qraw[:], 111, WW - 1, T, 1, HW))

                # sim = qk + A^T (psum) then + qr (DVE)
                s0 = ps_sim.tile([T, 448], f32, tag="sim0", name=f"s0_{b}_{I}")
                s1 = ps_sim.tile([T, 336], f32, tag="sim1", name=f"s1_{b}_{I}")
                nc.tensor.matmul(s0[:], lq, k_sb[b][:, 0:448],
                                 start=True, stop=False, skip_group_check=True)
                nc.tensor.matmul(s1[:], lq, k_sb[b][:, 448:HW],
                                 start=True, stop=False, skip_group_check=True)
                kr_ps = smt(T, WW, bf16, f"krp_{b}_{I}")
                for J in range(NT):
                    nc.tensor.matmul(
                        kr_ps[:, J * T:(J + 1) * T],
                        ash_f[J][:, I * T:(I + 1) * T],
                        idb_sb[0:T, 0:T],
                        is_transpose=True, start=True, stop=True,
                        skip_group_check=True)
                sim = wrk.tile([T, HW], f32, name="sim", tag="sim")
                nc.vector.tensor_tensor(sim[:, 0:448], s0[:], qsh[:, 0:448], OP.add)
                nc.vector.tensor_tensor(sim[:, 448:HW], s1[:], qsh[:, 448:HW], OP.add)
                nc.vector.tensor_tensor(sim[:, 0:448], sim[:, 0:448],
                                        kr_ps[:, 0:448], OP.add)
                nc.vector.tensor_tensor(sim[:, 448:HW], sim[:, 448:HW],
                                        kr_ps[:, 448:HW], OP.add)
                if _DEBUG and b == 0 and I == 0:
                    nc.sync.dma_start(dbg["sim0"][:], sim[:])

                # softmax (reversed-j storage, T-wide zero margins for shear)
                attn = wrk.tile([T, T + WW], bf16, name="attn", tag="attn", bufs=2)
                nc.any.memset(attn[:, 0:T], 0.0)
                nc.any.memset(attn[:, T + HW:T + WW], 0.0)
                nrm = wrk.tile([T, 1], f32, name="nrm", tag="nrm")
                rev = bass.AP(attn[:].tensor, attn[:].offset + T + HW - 1,
                              [[T + WW, T], [-1, HW]])
                nc.scalar.activation(rev, sim[:], AF.Exp,
                                     bias=nshift_sb[0:T, 0:1],
                                     accum_out=nrm[:])
                inv = wrk.tile([T, 1], f32, name="inv", tag="nrm")
                nc.vector.reciprocal(inv[:], nrm[:])
                nc.vector.tensor_scalar(attn[:, T:T + HW], attn[:, T:T + HW],
                                        inv[:], None, OP.mult)
                if _DEBUG and b == 0 and I == 0:
                    nc.gpsimd.dma_start(dbg["attn0"][:],
                                        attn[:, T:T + WW])

                # attnT chunks (descending j within chunk, matches vT)
                for J in range(NT):
                    pt = smt(T, T, bf16, f"atT_{b}_{I}_{J}")
                    nc.tensor.transpose(pt[:], attn[:, T + J * T:T + (J + 1) * T],
                                        idb_sb[0:T, 0:T])
                    nc.any.tensor_copy(attnT_sb[J][:, I * T:(I + 1) * T], pt[:])

                # sve: shear attn within tile, transpose chunks, contract
                ash = wrk.tile([T, WW], bf16, name="ash", tag="ash")
                nc.sync.dma_start(
                    ash[:], _diag(attn[:], T, T + WW - 1, T, 1, WW))
                if _DEBUG and b == 0 and I == 0:
                    nc.gpsimd.dma_start(dbg["ash0"][:], ash[:])
                sve_p = smt(VP, T, f32, f"svep_{b}_{I}")
                ashT = wrk.tile([T, WW], bf16, name="ashT", tag="ashT")
                for uc in range(8):
                    pt = smt(T, T, bf16, f"ashT_{b}_{I}_{uc}")
                    nc.tensor.transpose(pt[:], ash[:, uc * T:(uc + 1) * T],
                                        idb_sb[0:T, 0:T])
                    nc.any.tensor_copy(ashT[:, uc * T:(uc + 1) * T], pt[:])
                for uc in range(8):
                    s = I + uc
                    nc.tensor.matmul(sve_p[:], relvT_sb[:, s * VP:(s + 1) * VP],
                                     ashT[:, uc * T:(uc + 1) * T],
                                     start=(uc == 0), stop=(uc == 7))
                nc.scalar.activation(sve_sb[:, I * T:(I + 1) * T], sve_p[:],
                                     AF.Copy)

            # sv: attnT (j-chunks) x vT
            for J in range(NT):
                lhsT = vT_b[b][:, J * VP:(J + 1) * VP]
                nc.tensor.matmul(sv0[:], lhsT, attnT_sb[J][:, 0:448],
                                 start=(J == 0), stop=(J == 6))
                nc.tensor.matmul(sv1[:], lhsT, attnT_sb[J][:, 448:HW],
                                 start=(J == 0), stop=(J == 6))
            tb = wrk.tile([VP, HW], f32, name="tb", tag="tbin")
            nc.vector.tensor_tensor(tb[:, 0:448], sv0[:], sve_sb[:, 0:448], OP.add)
            nc.vector.tensor_tensor(tb[:, 448:HW], sv1[:], sve_sb[:, 448:HW], OP.add)
            nc.scalar.activation(bin_b[b][:], tb[:], AF.Identity,
                                 bias=bsum_sb[:, 0:1])
            nc.scalar.activation(rb_b[b][:], tb[:], AF.Relu, bias=bsum_sb[:, 0:1])
            if _DEBUG and b == 0:
                nc.sync.dma_start(dbg["bin0"][:], bin_b[0][:])
                nc.sync.dma_start(dbg["sve0"][:], sve_sb[:])
                sv_dbg = wrk.tile([VP, HW], f32, name="sv_dbg", tag="tbin")
                nc.vector.tensor_copy(sv_dbg[:, 0:448], sv0[:])
                nc.vector.tensor_copy(sv_dbg[:, 448:HW], sv1[:])
                nc.sync.dma_start(dbg["sv0"][:], sv_dbg[:])

        # ---------------- phase 2: 1x1 conv chain ----------------
        for b in range(NB):
            u0 = smt(VP, 448, f32, f"u0_{b}")
            u1p = smt(VP, 336, f32, f"u1_{b}")
            for kt in range(4):
                u3f = wrk.tile([128, HW], bf16, name="u3f", tag="u3f", bufs=3)
                for h in range(2):
                    r0_ = ((2 * kt + h) * NB + b) * VP
                    nc.sync.dma_start(u3f[h * VP:(h + 1) * VP, :],
                                      cc1_out[r0_:r0_ + VP, :])
                lhsT = wx1T_sb[:, kt * VP:(kt + 1) * VP]
                nc.tensor.matmul(u0[:], lhsT, u3f[:, 0:448],
                                 start=(kt == 0), stop=(kt == 3))
                nc.tensor.matmul(u1p[:], lhsT, u3f[:, 448:HW],
                                 start=(kt == 0), stop=(kt == 3))
            nc.scalar.activation(unary_b[b][:, 0:448], u0[:], AF.Identity,
                                 bias=bx_sb[:, 0:1])
            nc.scalar.activation(unary_b[b][:, 448:HW], u1p[:], AF.Identity,
                                 bias=bx_sb[:, 0:1])
            nc.scalar.activation(ru_b[b][:], unary_b[b][:], AF.Relu,
                                 bias=zero_sb[0:VP, 0:1])
            if _DEBUG and b == 0:
                nc.sync.dma_start(dbg["un0"][:], unary_b[0][:])
            for mt in range(4):
                r0 = smt(128, 448, f32, f"r0_{b}_{mt}")
                r1 = smt(128, 336, f32, f"r1_{b}_{mt}")
                lu = wruT_sb[:, mt * 128:(mt + 1) * 128]
                lb = wrbT_sb[:, mt * 128:(mt + 1) * 128]
                nc.tensor.matmul(r0[:], lu, ru_b[b][:, 0:448], start=True, stop=False)
                nc.tensor.matmul(r0[:], lb, rb_b[b][:, 0:448], start=False, stop=True)
                nc.tensor.matmul(r1[:], lu, ru_b[b][:, 448:HW], start=True, stop=False)
                nc.tensor.matmul(r1[:], lb, rb_b[b][:, 448:HW], start=False, stop=True)
                rp = wrk.tile([128, HW], bf16, name="rp", tag="rp")
                nc.scalar.activation(rp[:, 0:448], r0[:], AF.Copy)
                nc.scalar.activation(rp[:, 448:HW], r1[:], AF.Copy)
                nc.sync.dma_start(cc2_in[b][mt * 128:(mt + 1) * 128, :], rp[:])
            nc.gpsimd.collective_compute(
                "AllReduce", OP.add, replica_groups=[list(range(G))],
                ins=[cc2_in[b][:]], outs=[cc2_out[b][:]])
            g0 = smt(VP, 448, f32, f"g0_{b}")
            g1 = smt(VP, 336, f32, f"g1_{b}")
            for kt in range(4):
                rr = wrk.tile([128, HW], bf16, name="rr", tag="rr", bufs=3)
                nc.sync.dma_start(rr[:], cc2_out[b][kt * 128:(kt + 1) * 128, :])
                rrl = wrk.tile([128, HW], bf16, name="rrl", tag="rrl", bufs=3)
                nc.scalar.activation(rrl[:], rr[:], AF.Relu,
                                     bias=br_sb[:, kt:kt + 1])
                lhsT = wpT_sb[:, kt * VP:(kt + 1) * VP]
                nc.tensor.matmul(g0[:], lhsT, rrl[:, 0:448],
                                 start=(kt == 0), stop=(kt == 3))
                nc.tensor.matmul(g1[:], lhsT, rrl[:, 448:HW],
                                 start=(kt == 0), stop=(kt == 3))
            gate = wrk.tile([VP, HW], f32, name="gate", tag="gate")
            nc.scalar.activation(gate[:, 0:448], g0[:], AF.Sigmoid,
                                 bias=bp_sb[:, 0:1])
            nc.scalar.activation(gate[:, 448:HW], g1[:], AF.Sigmoid,
                                 bias=bp_sb[:, 0:1])
            if _DEBUG and b == 0:
                nc.sync.dma_start(dbg["gate0"][:], gate[:])
            nc.vector.tensor_tensor(gate[:], gate[:], bin_b[b][:], OP.mult)
            nc.vector.tensor_tensor(gate[:], gate[:], unary_b[b][:], OP.add)
            nc.sync.dma_start(out_d[b], gate[:])

    _split_multiwaits(nc)
    return nc


_CACHED_NC = None


def _host_prep(x, W_qkv, g_qkv, b_qkv, relative, g_sim, b_sim, g_out, b_out,
               W_x3, W_x1, g_x, b_x, W_r, g_r, b_r, W_p, g_p, b_p):
    sc = 1.0 / np.sqrt(1.0 + EPS)
    s_qkv = (np.float32(g_qkv) * sc).astype(np.float32)
    bq = np.float32(b_qkv)
    s_sim = (np.float32(g_sim) * sc).astype(np.float32)
    s_out = (np.float32(g_out) * sc).astype(np.float32).reshape(G, VP, 2)
    bo = np.float32(b_out).reshape(G, VP, 2)
    s_x = (np.float32(g_x) * sc); s_r = (np.float32(g_r) * sc)
    s_p = (np.float32(g_p) * sc)
    Wq = np.float32(W_qkv)
    xf = np.ascontiguousarray(np.float32(x).reshape(NB, BR, HW))
    rel = np.float32(relative)
    W3 = np.float32(W_x3).reshape(G, VP, VP, 3, 3)
    Wx1 = np.float32(W_x1)[:, :, 0, 0]
    Wr = np.float32(W_r)[:, :, 0, 0]
    Wp = np.float32(W_p)[:, :, 0, 0]
    b_x = np.float32(b_x); b_r = np.float32(b_r); b_p = np.float32(b_p)

    in_maps = []
    for g in range(G):
        s0, s1, s2 = s_sim[g], s_sim[G + g], s_sim[2 * G + g]
        a = np.sign(s0) * np.sqrt(max(abs(s0), 1e-30))
        be = np.sqrt(max(abs(s0), 1e-30))
        rows = np.empty(128, np.float32)
        rows[0:QK] = a
        rows[QK:2 * QK] = be
        rows[2 * QK:] = s_out[g, :, 0]
        base = 128 * g
        Wg = Wq[base:base + 128] * (rows * s_qkv[base:base + 128])[:, None]
        bqg = (bq[base:base + 128] * rows).astype(np.float32)[:, None]
        wqkvT = np.ascontiguousarray(
            Wg.T.reshape(4, 128, 128)).astype(np.float32)   # [kt, cc, o]
        relq_rev = (rel[0:QK, ::-1] * (s1 / a)).astype(np.float32)
        relk_rev = (rel[QK:2 * QK, ::-1] * (s2 / be)).astype(np.float32)
        relvT = np.zeros((2 * HW, VP), np.float32)
        relvT[0:2 * HW - 1] = rel[2 * QK:].T * s_out[g, :, 1][None, :]
        w3T = np.ascontiguousarray(
            W3[g].transpose(2, 3, 1, 0).reshape(9, VP, VP)).astype(BF)
        wx1T = np.ascontiguousarray(
            (Wx1[VP * g:VP * (g + 1)] * s_x[VP * g:VP * (g + 1), None]).T
            .reshape(4, 128, VP)).astype(BF)
        wruT = np.ascontiguousarray(
            (Wr[:, VP * g:VP * (g + 1)] * s_r[:, None]).T).astype(BF)
        wrbT = np.ascontiguousarray(
            (Wr[:, BR + VP * g:BR + VP * (g + 1)] * s_r[:, None]).T).astype(BF)
        wpT = np.ascontiguousarray(
            (Wp[VP * g:VP * (g + 1)] * s_p[VP * g:VP * (g + 1), None]).T
            .reshape(4, 128, VP)).astype(BF)
        in_maps.append({
            "xf": xf,
            "xg": np.ascontiguousarray(
                np.float32(x).reshape(NB, BR, HW)[:, VP * g:VP * (g + 1)]).astype(BF),
            "wqkvT": wqkvT,
            "bqkv": bqg,
            "relq": np.ascontiguousarray(relq_rev),
            "relk": np.ascontiguousarray(relk_rev),
            "relvT": relvT.astype(BF),
            "w3T": w3T,
            "wx1T": wx1T,
            "bx": b_x[VP * g:VP * (g + 1), None].astype(np.float32),
            "wruT": wruT,
            "wrbT": wrbT,
            "br": np.ascontiguousarray(b_r.reshape(4, 128).T).astype(np.float32),
            "wpT": wpT,
            "bp": b_p[VP * g:VP * (g + 1), None].astype(np.float32),
            "bsum": (bo[g, :, 0] + bo[g, :, 1])[:, None].astype(np.float32),
            "identb": np.eye(128, dtype=BF),
            "identf": np.eye(128, dtype=np.float32),
        })
    return in_maps


def run(inputs, trace=False):
    global _CACHED_NC
    _install_profile_hook()
    from concourse.bass_utils import run_bass_kernel_spmd
    if _CACHED_NC is None:
        _CACHED_NC = _build()
    in_maps = _host_prep(**inputs)
    res = run_bass_kernel_spmd(_CACHED_NC, in_maps, list(range(G)), trace=trace)
    out = np.concatenate(
        [res.results[g]["out"].reshape(NB, VP, 28, 28) for g in range(G)], axis=1)
    return out.astype(np.float32), res


def kernel(**inputs):
    out, _ = run(inputs, trace=False)
    return out


# revision 12
# speedup vs baseline: 1.0777x; 1.0777x over previous
"""nn_LESA Trainium2 kernel: 8-core SPMD Bass/Tile implementation.

Sharding: attention-group parallel (core g owns head-group g for all 4
batches: qkv projection slice, relative-position attention, grouped-conv
group, and the 64-channel output slice). Two small collectives knit the
full-channel 1x1-conv chain together: an AllGather of the grouped-conv
output and a per-batch AllReduce of the W_r partial products.

The relative-position einsums (qr/kr/sve) are computed as dense GEMMs
against `relative` in (i, t=i-j+HW-1) coordinates, then mapped back to
(i, j) with diagonal SBUF->SBUF DMA access patterns (per-partition
shifted windows); kr additionally needs PE transposes, as does attn for
the sv/sve contractions.
"""
import os
import sys
import types

import numpy as np
import ml_dtypes

import concourse.bass as bass
from concourse import mybir
from concourse.tile import TileContext

G = 8
NB = 4          # batches
QK = 32
VP = 64
HW = 784
T = 112         # pixel tile
NT = 7          # HW // T
WW = 896        # padded shear window width (8*T)
BR = 512
EPS = 1e-5
SHIFT = 20.0    # exp(x - SHIFT); cancels in normalization

f32 = mybir.dt.float32
bf16 = mybir.dt.bfloat16
BF = ml_dtypes.bfloat16

_DEBUG = bool(int(os.environ.get("BASSK_DEBUG", "0")))


def _split_multiwaits(nc):
    """walrus in this env allows 1 sync-wait per instruction; hoist extras
    onto same-engine NOPs placed just before the waiting instruction."""
    for f in nc.m.functions:
        for b in f.blocks:
            out = []
            for inst in b.instructions:
                si = inst.sync_info
                if si is not None and len(si.on_wait) > 1:
                    waits = list(si.on_wait)
                    for k, w in enumerate(waits[:-1]):
                        nop = mybir.InstNoOp(name=f"{inst.name}_w{k}", ins=[], outs=[])
                        nop.engine = inst.engine
                        nop.sync_info = mybir.SyncInfo(on_wait=[w], on_update=[])
                        out.append(nop)
                    inst.sync_info = mybir.SyncInfo(
                        on_wait=waits[-1:], on_update=list(si.on_update))
                out.append(inst)
            b.instructions = out


def _install_profile_hook():
    """antenv.axon_hooks is absent in this image; synthesize it so
    run_bass_kernel_spmd(trace=True) can reach the ctypes NTFF hook."""
    if "antenv.axon_hooks" in sys.modules:
        return
    try:
        from trn_agent_boot.trn_boot import _ntff_profile_via_ctypes
        hook = _ntff_profile_via_ctypes("/opt/axon/libaxon_pjrt.so")
    except Exception:
        hook = None
    mod = types.ModuleType("antenv.axon_hooks")
    mod.get_axon_ntff_profile_hook = lambda: hook
    mod.set_axon_ntff_profile_hook = lambda h: None
    sys.modules["antenv.axon_hooks"] = mod


def _diag(t_ap, extra_off, pstep, count, fstep, fcount):
    return bass.AP(t_ap.tensor, t_ap.offset + extra_off,
                   [[pstep, count], [fstep, fcount]])


def _build():
    nc = bass.Bass(num_devices=G)
    dt_in = {}

    def inp(name, shape, dt):
        dt_in[name] = nc.dram_tensor(name, list(shape), dt, kind="ExternalInput")
        return dt_in[name]

    xf = inp("xf", (NB, BR, HW), f32)
    xg = inp("xg", (NB, VP, HW), bf16)
    wqkvT = inp("wqkvT", (4, 128, 128), f32)
    bqkv = inp("bqkv", (128, 1), f32)
    relq = inp("relq", (QK, 2 * HW - 1), f32)
    relk = inp("relk", (QK, 2 * HW - 1), f32)
    relvT = inp("relvT", (2 * HW, VP), bf16)      # row 2*HW-1 zero-padded
    w3T = inp("w3T", (9, VP, VP), bf16)
    wx1T = inp("wx1T", (4, 128, VP), bf16)
    bx = inp("bx", (VP, 1), f32)
    wruT = inp("wruT", (VP, BR), bf16)
    wrbT = inp("wrbT", (VP, BR), bf16)
    br = inp("br", (128, 4), f32)
    wpT = inp("wpT", (4, 128, VP), bf16)
    bp = inp("bp", (VP, 1), f32)
    bsum = inp("bsum", (VP, 1), f32)
    identb = inp("identb", (128, 128), bf16)
    identf = inp("identf", (128, 128), f32)

    out_d = nc.dram_tensor("out", [NB, VP, HW], f32, kind="ExternalOutput")
    dbg = {}
    if _DEBUG:
        dbg["qkv0"] = nc.dram_tensor("dbg_qkv0", [128, HW], f32, kind="ExternalOutput")
        dbg["sim0"] = nc.dram_tensor("dbg_sim0", [T, HW], f32, kind="ExternalOutput")
        dbg["attn0"] = nc.dram_tensor("dbg_attn0", [T, WW], f32, kind="ExternalOutput")
        dbg["ash0"] = nc.dram_tensor("dbg_ash0", [T, WW], f32, kind="ExternalOutput")
        dbg["bin0"] = nc.dram_tensor("dbg_bin0", [VP, HW], f32, kind="ExternalOutput")
        dbg["u30"] = nc.dram_tensor("dbg_u30", [VP, HW], f32, kind="ExternalOutput")
        dbg["un0"] = nc.dram_tensor("dbg_un0", [VP, HW], f32, kind="ExternalOutput")
        dbg["gate0"] = nc.dram_tensor("dbg_gate0", [VP, HW], f32, kind="ExternalOutput")
        dbg["sve0"] = nc.dram_tensor("dbg_sve0", [VP, HW], f32, kind="ExternalOutput")
        dbg["sv0"] = nc.dram_tensor("dbg_sv0", [VP, HW], f32, kind="ExternalOutput")

    AF = mybir.ActivationFunctionType
    OP = mybir.AluOpType

    from contextlib import ExitStack
    with TileContext(nc) as tc, ExitStack() as stk:
        cst = stk.enter_context(tc.tile_pool(name="cst", bufs=1))
        wrk = stk.enter_context(tc.tile_pool(name="wrk", bufs=2))
        drp = stk.enter_context(tc.tile_pool(name="drp", bufs=1, space="DRAM"))
        # PSUM: sim (4 banks) + small pool (4 banks) = 8
        ps_sim = stk.enter_context(tc.tile_pool(name="ps_sim", bufs=4, space="PSUM"))
        ps_sm = stk.enter_context(tc.tile_pool(name="ps_sm", bufs=4, space="PSUM"))

        def smt(p, q, dt, name):
            return ps_sm.tile([p, q], dt, tag="sm", name=name)

        # ---------------- constants into SBUF ----------------
        xpad = cst.tile([VP, NB * 900], bf16, name="xpad")
        nc.gpsimd.memset(xpad[:], 0.0)
        for b in range(NB):
            dst = bass.AP(xpad[:].tensor, xpad[:].offset + b * 900 + 31,
                          [[NB * 900, VP], [30, 28], [1, 28]])
            nc.sync.dma_start(dst, xg[b])
        wqkvT_sb = cst.tile([128, 512], f32, name="wqkvT_sb")
        nc.sync.dma_start(
            wqkvT_sb[:].rearrange("p (k f) -> p k f", k=4),
            bass.AP(wqkvT, 0, [[128, 128], [128 * 128, 4], [1, 128]]))
        bqkv_sb = cst.tile([128, 1], f32, name="bqkv_sb")
        nc.sync.dma_start(bqkv_sb[:], bqkv[:])
        relq_sb = cst.tile([QK, 2 * HW - 1], f32, name="relq_sb")
        nc.sync.dma_start(relq_sb[:], relq[:])
        relk_sb = cst.tile([QK, 2 * HW - 1], f32, name="relk_sb")
        nc.sync.dma_start(relk_sb[:], relk[:])
        relvT_sb = cst.tile([T, 14 * VP], bf16, name="relvT_sb")
        nc.sync.dma_start(
            relvT_sb[:].rearrange("p (k f) -> p k f", k=14),
            bass.AP(relvT, 0, [[VP, T], [T * VP, 14], [1, VP]]))
        w3T_sb = cst.tile([VP, 9 * VP], bf16, name="w3T_sb")
        nc.sync.dma_start(
            w3T_sb[:].rearrange("p (k f) -> p k f", k=9),
            bass.AP(w3T, 0, [[VP, VP], [VP * VP, 9], [1, VP]]))
        wx1T_sb = cst.tile([128, 4 * VP], bf16, name="wx1T_sb")
        nc.sync.dma_start(
            wx1T_sb[:].rearrange("p (k f) -> p k f", k=4),
            bass.AP(wx1T, 0, [[VP, 128], [128 * VP, 4], [1, VP]]))
        wruT_sb = cst.tile([VP, BR], bf16, name="wruT_sb")
        nc.sync.dma_start(wruT_sb[:], wruT[:])
        wrbT_sb = cst.tile([VP, BR], bf16, name="wrbT_sb")
        nc.sync.dma_start(wrbT_sb[:], wrbT[:])
        wpT_sb = cst.tile([128, 4 * VP], bf16, name="wpT_sb")
        nc.sync.dma_start(
            wpT_sb[:].rearrange("p (k f) -> p k f", k=4),
            bass.AP(wpT, 0, [[VP, 128], [128 * VP, 4], [1, VP]]))
        bx_sb = cst.tile([VP, 1], f32, name="bx_sb")
        nc.sync.dma_start(bx_sb[:], bx[:])
        br_sb = cst.tile([128, 4], f32, name="br_sb")
        nc.sync.dma_start(br_sb[:], br[:])
        bp_sb = cst.tile([VP, 1], f32, name="bp_sb")
        nc.sync.dma_start(bp_sb[:], bp[:])
        bsum_sb = cst.tile([VP, 1], f32, name="bsum_sb")
        nc.sync.dma_start(bsum_sb[:], bsum[:])
        idb_sb = cst.tile([128, 128], bf16, name="idb_sb")
        nc.sync.dma_start(idb_sb[:], identb[:])
        idf_sb = cst.tile([128, 128], f32, name="idf_sb")
        nc.sync.dma_start(idf_sb[:], identf[:])
        zero_sb = cst.tile([128, 1], f32, name="zero_sb")
        nc.gpsimd.memset(zero_sb[:], 0.0)
        nshift_sb = cst.tile([128, 1], f32, name="nshift_sb")
        nc.gpsimd.memset(nshift_sb[:], -SHIFT)

        # persistent per-batch tiles
        q_sb = [cst.tile([QK, HW], f32, name=f"q{b}") for b in range(NB)]
        k_sb = [cst.tile([QK, HW], f32, name=f"k{b}") for b in range(NB)]
        v_sb = [cst.tile([VP, HW], bf16, name=f"v{b}") for b in range(NB)]
        vT_b = [cst.tile([T, NT * VP], bf16, name=f"vT{b}") for b in range(NB)]
        unary_b = [cst.tile([VP, HW], f32, name=f"un{b}") for b in range(NB)]
        ru_b = [cst.tile([VP, HW], bf16, name=f"ru{b}") for b in range(NB)]
        bin_b = [cst.tile([VP, HW], f32, name=f"bin{b}") for b in range(NB)]
        rb_b = [cst.tile([VP, HW], bf16, name=f"rb{b}") for b in range(NB)]

        # collective buffers (DRAM pool tiles so Tile tracks deps)
        cc1_in = drp.tile([NB * VP, HW], bf16, name="cc1_in")
        cc1_out = drp.tile([G * NB * VP, HW], bf16, name="cc1_out",
                           addr_space="Shared")
        cc2_in = [drp.tile([BR, HW], bf16, name=f"cc2i{b}") for b in range(NB)]
        cc2_out = [drp.tile([BR, HW], bf16, name=f"cc2o{b}",
                            addr_space="Shared") for b in range(NB)]

        NCH = (448, 336)  # HW split, bank-aligned psum chunks

        # ---------------- qkv projection + conv3x3 (all batches) ------------
        for b in range(NB):
            p0 = smt(128, 392, f32, f"qv0_{b}")
            p1 = smt(128, 392, f32, f"qv1_{b}")
            for kt in range(4):
                rhs = wrk.tile([128, HW], f32, name="xft", tag="xft", bufs=2)
                nc.sync.dma_start(
                    rhs[:], bass.AP(xf, b * BR * HW + kt * 128 * HW,
                                    [[HW, 128], [1, HW]]))
                lhsT = wqkvT_sb[:, kt * 128:(kt + 1) * 128]
                nc.tensor.matmul(p0[:], lhsT, rhs[:, 0:392],
                                 start=(kt == 0), stop=(kt == 3))
                nc.tensor.matmul(p1[:], lhsT, rhs[:, 392:HW],
                                 start=(kt == 0), stop=(kt == 3))
            qkv_f = wrk.tile([128, HW], f32, name="qkv_f", tag="qkv_f")
            nc.scalar.activation(qkv_f[:, 0:392], p0[:], AF.Identity,
                                 bias=bqkv_sb[:, 0:1])
            nc.scalar.activation(qkv_f[:, 392:HW], p1[:], AF.Identity,
                                 bias=bqkv_sb[:, 0:1])
            # partition-rebase q/k/v to base 0 (matmul needs equal bases)
            nc.sync.dma_start(q_sb[b][:], qkv_f[0:QK, :])
            nc.sync.dma_start(k_sb[b][:], qkv_f[QK:2 * QK, :])
            nc.gpsimd.dma_start(v_sb[b][:], qkv_f[2 * QK:128, :])
            # v reversed copy then plain transposes -> vT (descending j chunks)
            vrev = wrk.tile([VP, HW], bf16, name="vrev", tag="vrev")
            rev_out = bass.AP(vrev[:].tensor, vrev[:].offset + HW - 1,
                              [[HW, VP], [-1, HW]])
            nc.scalar.activation(rev_out, v_sb[b][:], AF.Copy)
            for j in range(NT):
                pt = smt(T, VP, bf16, f"vtp_{b}_{j}")
                nc.tensor.transpose(pt[:], vrev[:, j * T:(j + 1) * T],
                                    idb_sb[0:VP, 0:VP])
                nc.vector.tensor_copy(vT_b[b][:, j * VP:(j + 1) * VP], pt[:])
            # grouped 3x3 conv (unary branch pre-1x1)
            c0 = smt(VP, 392, f32, f"cv0_{b}")
            c1 = smt(VP, 392, f32, f"cv1_{b}")
            for k in range(9):
                dy, dx = divmod(k, 3)
                lhsT = w3T_sb[:, k * VP:(k + 1) * VP]
                for h, cp in ((0, c0), (1, c1)):
                    rhs = bass.AP(xpad[:].tensor,
                                  xpad[:].offset + b * 900 + dy * 30 + dx + h * 420,
                                  [[NB * 900, VP], [30, 14], [1, 28]])
                    nc.tensor.matmul(cp[:], lhsT, rhs,
                                     start=(k == 0), stop=(k == 8))
            u3 = wrk.tile([VP, HW], bf16, name="u3", tag="u3")
            nc.scalar.activation(u3[:, 0:392], c0[:], AF.Copy)
            nc.scalar.activation(u3[:, 392:HW], c1[:], AF.Copy)
            if _DEBUG and b == 0:
                nc.gpsimd.dma_start(dbg["u30"][:], u3[:])
                pass
            nc.sync.dma_start(cc1_in[b * VP:(b + 1) * VP, :], u3[:])

        nc.gpsimd.collective_compute(
            "AllGather", OP.bypass, replica_groups=[list(range(G))],
            ins=[cc1_in[:]], outs=[cc1_out[:]])

        # ---------------- attention + phase-2 part 1, per batch ----------
        for b in range(NB):
            # kr precompute: a_t_rev GEMM windows + shear -> A[j, i] rows
            ash_f = []
            for J in range(NT):
                w0 = 672 - J * T
                a0 = smt(T, 448, f32, f"at0_{b}_{J}")
                a1 = smt(T, 447, f32, f"at1_{b}_{J}")
                lhsT = k_sb[b][:, J * T:(J + 1) * T]
                nc.tensor.matmul(a0[:], lhsT, relk_sb[:, w0:w0 + 448],
                                 start=True, stop=True)
                nc.tensor.matmul(a1[:], lhsT, relk_sb[:, w0 + 448:w0 + 895],
                                 start=True, stop=True)
                araw = wrk.tile([T, WW], bf16, name="araw", tag="araw", bufs=3)
                nc.scalar.activation(araw[:, 0:448], a0[:], AF.Copy)
                nc.scalar.activation(araw[:, 448:895], a1[:], AF.Copy)
                af = wrk.tile([T, HW], bf16, name=f"ashf{J}", tag=f"ashf{J}",
                              bufs=2)
                nc.sync.dma_start(af[:], _diag(araw[:], 111, WW - 1, T, 1, HW))
                ash_f.append(af)

            for I in range(NT):
                w0 = 672 - I * T
                # qr GEMM + shear
                q0 = smt(T, 448, f32, f"qr0_{b}_{I}")
                q1 = smt(T, 447, f32, f"qr1_{b}_{I}")
                lq = q_sb[b][:, I * T:(I + 1) * T]
                nc.tensor.matmul(q0[:], lq, relq_sb[:, w0:w0 + 448],
                                 start=True, stop=True)
                nc.tensor.matmul(q1[:], lq, relq_sb[:, w0 + 448:w0 + 895],
                                 start=True, stop=True)
                qraw = wrk.tile([T, WW], bf16, name="qraw", tag="qraw")
                nc.scalar.activation(qraw[:, 0:448], q0[:], AF.Copy)
                nc.scalar.activation(qraw[:, 448:895], q1[:], AF.Copy)
                qsh = wrk.tile([T, HW], bf16, name="qsh", tag="qsh")
                nc.sync.dma_start(qsh[:], _diag(qraw[:], 111, WW - 1, T, 1, HW))

                # sim = qk (psum) + kr (transposed psum) + qr (DVE)
                s0 = ps_sim.tile([T, 392], f32, tag="sim", name=f"s0_{b}_{I}")
                s1 = ps_sim.tile([T, 392], f32, tag="sim", name=f"s1_{b}_{I}")
                nc.tensor.matmul(s0[:], lq, k_sb[b][:, 0:392],
                                 start=True, stop=True, skip_group_check=True)
                nc.tensor.matmul(s1[:], lq, k_sb[b][:, 392:HW],
                                 start=True, stop=True, skip_group_check=True)
                kr_ps = smt(T, WW, bf16, f"krp_{b}_{I}")
                for J in range(NT):
                    nc.tensor.matmul(
                        kr_ps[:, J * T:(J + 1) * T],
                        ash_f[J][:, I * T:(I + 1) * T],
                        idb_sb[0:T, 0:T],
                        is_transpose=True, start=True, stop=True,
                        skip_group_check=True)
                sim = wrk.tile([T, HW], f32, name="sim", tag="sim")
                nc.vector.tensor_tensor(sim[:, 0:392], s0[:], qsh[:, 0:392], OP.add)
                nc.vector.tensor_tensor(sim[:, 392:HW], s1[:], qsh[:, 392:HW], OP.add)
                nc.vector.tensor_tensor(sim[:, 0:392], sim[:, 0:392],
                                        kr_ps[:, 0:392], OP.add)
                nc.vector.tensor_tensor(sim[:, 392:HW], sim[:, 392:HW],
                                        kr_ps[:, 392:HW], OP.add)
                if _DEBUG and b == 0 and I == 0:
                    nc.sync.dma_start(dbg["sim0"][:], sim[:])

                # softmax (reversed-j storage, T-wide zero margins for shear)
                attn = wrk.tile([T, T + WW], bf16, name="attn", tag="attn", bufs=2)
                nc.any.memset(attn[:, 0:T], 0.0)
                nc.any.memset(attn[:, T + HW:T + WW], 0.0)
                nrm = wrk.tile([T, 1], f32, name="nrm", tag="nrm")
                rev = bass.AP(attn[:].tensor, attn[:].offset + T + HW - 1,
                              [[T + WW, T], [-1, HW]])
                nc.scalar.activation(rev, sim[:], AF.Exp,
                                     bias=nshift_sb[0:T, 0:1],
                                     accum_out=nrm[:])
                inv = wrk.tile([T, 1], f32, name="inv", tag="nrm")
                nc.vector.reciprocal(inv[:], nrm[:])
                nc.vector.tensor_scalar(attn[:, T:T + HW], attn[:, T:T + HW],
                                        inv[:], None, OP.mult)
                if _DEBUG and b == 0 and I == 0:
                    nc.gpsimd.dma_start(dbg["attn0"][:], attn[:, T:T + WW])

                # attnT chunks (descending j within chunk, matches vT)
                attnT = wrk.tile([T, HW], bf16, name="attnT", tag="attnT", bufs=2)
                for J in range(NT):
                    pt = smt(T, T, bf16, f"atT_{b}_{I}_{J}")
                    nc.tensor.transpose(pt[:], attn[:, T + J * T:T + (J + 1) * T],
                                        idb_sb[0:T, 0:T])
                    nc.vector.tensor_copy(attnT[:, J * T:(J + 1) * T], pt[:])

                # sve shear + transposed chunks
                ash = wrk.tile([T, WW], bf16, name="ash", tag="ash")
                nc.sync.dma_start(
                    ash[:], _diag(attn[:], T, T + WW - 1, T, 1, WW))
                ashT = wrk.tile([T, WW], bf16, name="ashT", tag="ashT")
                for uc in range(8):
                    pt = smt(T, T, bf16, f"ashT_{b}_{I}_{uc}")
                    nc.tensor.transpose(pt[:], ash[:, uc * T:(uc + 1) * T],
                                        idb_sb[0:T, 0:T])
                    nc.vector.tensor_copy(ashT[:, uc * T:(uc + 1) * T], pt[:])

                # combined sv + sve accumulation -> binary slice
                bp_ps = smt(VP, T, f32, f"bin_{b}_{I}")
                for uc in range(8):
                    s = I + uc
                    nc.tensor.matmul(bp_ps[:], relvT_sb[:, s * VP:(s + 1) * VP],
                                     ashT[:, uc * T:(uc + 1) * T],
                                     start=(uc == 0), stop=False,
                                     skip_group_check=True)
                for J in range(NT):
                    nc.tensor.matmul(bp_ps[:], vT_b[b][:, J * VP:(J + 1) * VP],
                                     attnT[:, J * T:(J + 1) * T],
                                     start=False, stop=(J == NT - 1),
                                     skip_group_check=True)
                nc.scalar.activation(bin_b[b][:, I * T:(I + 1) * T], bp_ps[:],
                                     AF.Identity, bias=bsum_sb[:, 0:1])
                nc.scalar.activation(rb_b[b][:, I * T:(I + 1) * T], bp_ps[:],
                                     AF.Relu, bias=bsum_sb[:, 0:1])
            if _DEBUG and b == 0:
                nc.sync.dma_start(dbg["bin0"][:], bin_b[0][:])

            # ---- phase 2 part 1: unary 1x1, W_r partials, AllReduce issue ----
            u0 = smt(VP, 392, f32, f"u0_{b}")
            u1p = smt(VP, 392, f32, f"u1_{b}")
            for kt in range(4):
                u3f = wrk.tile([128, HW], bf16, name="u3f", tag="u3f", bufs=3)
                for h in range(2):
                    r0_ = ((2 * kt + h) * NB + b) * VP
                    nc.sync.dma_start(u3f[h * VP:(h + 1) * VP, :],
                                      cc1_out[r0_:r0_ + VP, :])
                lhsT = wx1T_sb[:, kt * VP:(kt + 1) * VP]
                nc.tensor.matmul(u0[:], lhsT, u3f[:, 0:392],
                                 start=(kt == 0), stop=(kt == 3))
                nc.tensor.matmul(u1p[:], lhsT, u3f[:, 392:HW],
                                 start=(kt == 0), stop=(kt == 3))
            nc.scalar.activation(unary_b[b][:, 0:392], u0[:], AF.Identity,
                                 bias=bx_sb[:, 0:1])
            nc.scalar.activation(unary_b[b][:, 392:HW], u1p[:], AF.Identity,
                                 bias=bx_sb[:, 0:1])
            nc.scalar.activation(ru_b[b][:], unary_b[b][:], AF.Relu,
                                 bias=zero_sb[0:VP, 0:1])
            if _DEBUG and b == 0:
                nc.sync.dma_start(dbg["un0"][:], unary_b[0][:])
            for mt in range(4):
                r0 = smt(128, 392, f32, f"r0_{b}_{mt}")
                r1 = smt(128, 392, f32, f"r1_{b}_{mt}")
                lu = wruT_sb[:, mt * 128:(mt + 1) * 128]
                lb = wrbT_sb[:, mt * 128:(mt + 1) * 128]
                nc.tensor.matmul(r0[:], lu, ru_b[b][:, 0:392], start=True, stop=False)
                nc.tensor.matmul(r0[:], lb, rb_b[b][:, 0:392], start=False, stop=True)
                nc.tensor.matmul(r1[:], lu, ru_b[b][:, 392:HW], start=True, stop=False)
                nc.tensor.matmul(r1[:], lb, rb_b[b][:, 392:HW], start=False, stop=True)
                rp = wrk.tile([128, HW], bf16, name="rp", tag="rp")
                nc.scalar.activation(rp[:, 0:392], r0[:], AF.Copy)
                nc.scalar.activation(rp[:, 392:HW], r1[:], AF.Copy)
                nc.sync.dma_start(cc2_in[b][mt * 128:(mt + 1) * 128, :], rp[:])
            nc.gpsimd.collective_compute(
                "AllReduce", OP.add, replica_groups=[list(range(G))],
                ins=[cc2_in[b][:]], outs=[cc2_out[b][:]])

        # ---------------- phase 2 part 2: gate + output ----------------
        for b in range(NB):
            g0 = smt(VP, 392, f32, f"g0_{b}")
            g1 = smt(VP, 392, f32, f"g1_{b}")
            for kt in range(4):
                rr = wrk.tile([128, HW], bf16, name="rr", tag="rr", bufs=3)
                nc.sync.dma_start(rr[:], cc2_out[b][kt * 128:(kt + 1) * 128, :])
                rrl = wrk.tile([128, HW], bf16, name="rrl", tag="rrl", bufs=3)
                nc.scalar.activation(rrl[:], rr[:], AF.Relu,
                                     bias=br_sb[:, kt:kt + 1])
                lhsT = wpT_sb[:, kt * VP:(kt + 1) * VP]
                nc.tensor.matmul(g0[:], lhsT, rrl[:, 0:392],
                                 start=(kt == 0), stop=(kt == 3))
                nc.tensor.matmul(g1[:], lhsT, rrl[:, 392:HW],
                                 start=(kt == 0), stop=(kt == 3))
            gate = wrk.tile([VP, HW], f32, name="gate", tag="gate")
            nc.scalar.activation(gate[:, 0:392], g0[:], AF.Sigmoid,
                                 bias=bp_sb[:, 0:1])
            nc.scalar.activation(gate[:, 392:HW], g1[:], AF.Sigmoid,
                                 bias=bp_sb[:, 0:1])
            if _DEBUG and b == 0:
                nc.sync.dma_start(dbg["gate0"][:], gate[:])
            nc.vector.tensor_tensor(gate[:], gate[:], bin_b[b][:], OP.mult)
            nc.vector.tensor_tensor(gate[:], gate[:], unary_b[b][:], OP.add)
            nc.sync.dma_start(out_d[b], gate[:])

    _split_multiwaits(nc)
    return nc


_CACHED_NC = None


def _host_prep(x, W_qkv, g_qkv, b_qkv, relative, g_sim, b_sim, g_out, b_out,
               W_x3, W_x1, g_x, b_x, W_r, g_r, b_r, W_p, g_p, b_p):
    sc = 1.0 / np.sqrt(1.0 + EPS)
    s_qkv = (np.float32(g_qkv) * sc).astype(np.float32)
    bq = np.float32(b_qkv)
    s_sim = (np.float32(g_sim) * sc).astype(np.float32)
    s_out = (np.float32(g_out) * sc).astype(np.float32).reshape(G, VP, 2)
    bo = np.float32(b_out).reshape(G, VP, 2)
    s_x = (np.float32(g_x) * sc); s_r = (np.float32(g_r) * sc)
    s_p = (np.float32(g_p) * sc)
    Wq = np.float32(W_qkv)
    xf = np.ascontiguousarray(np.float32(x).reshape(NB, BR, HW))
    rel = np.float32(relative)
    W3 = np.float32(W_x3).reshape(G, VP, VP, 3, 3)
    Wx1 = np.float32(W_x1)[:, :, 0, 0]
    Wr = np.float32(W_r)[:, :, 0, 0]
    Wp = np.float32(W_p)[:, :, 0, 0]
    b_x = np.float32(b_x); b_r = np.float32(b_r); b_p = np.float32(b_p)

    in_maps = []
    for g in range(G):
        s0, s1, s2 = s_sim[g], s_sim[G + g], s_sim[2 * G + g]
        a = np.sign(s0) * np.sqrt(max(abs(s0), 1e-30))
        be = np.sqrt(max(abs(s0), 1e-30))
        rows = np.empty(128, np.float32)
        rows[0:QK] = a
        rows[QK:2 * QK] = be
        rows[2 * QK:] = s_out[g, :, 0]
        base = 128 * g
        Wg = Wq[base:base + 128] * (rows * s_qkv[base:base + 128])[:, None]
        bqg = (bq[base:base + 128] * rows).astype(np.float32)[:, None]
        wqkvT = np.ascontiguousarray(
            Wg.T.reshape(4, 128, 128)).astype(np.float32)   # [kt, cc, o]
        relq_rev = (rel[0:QK, ::-1] * (s1 / a)).astype(np.float32)
        relk_rev = (rel[QK:2 * QK, ::-1] * (s2 / be)).astype(np.float32)
        relvT = np.zeros((2 * HW, VP), np.float32)
        relvT[0:2 * HW - 1] = rel[2 * QK:].T * s_out[g, :, 1][None, :]
        w3T = np.ascontiguousarray(
            W3[g].transpose(2, 3, 1, 0).reshape(9, VP, VP)).astype(BF)
        wx1T = np.ascontiguousarray(
            (Wx1[VP * g:VP * (g + 1)] * s_x[VP * g:VP * (g + 1), None]).T
            .reshape(4, 128, VP)).astype(BF)
        wruT = np.ascontiguousarray(
            (Wr[:, VP * g:VP * (g + 1)] * s_r[:, None]).T).astype(BF)
        wrbT = np.ascontiguousarray(
            (Wr[:, BR + VP * g:BR + VP * (g + 1)] * s_r[:, None]).T).astype(BF)
        wpT = np.ascontiguousarray(
            (Wp[VP * g:VP * (g + 1)] * s_p[VP * g:VP * (g + 1), None]).T
            .reshape(4, 128, VP)).astype(BF)
        in_maps.append({
            "xf": xf,
            "xg": np.ascontiguousarray(
                np.float32(x).reshape(NB, BR, HW)[:, VP * g:VP * (g + 1)]).astype(BF),
            "wqkvT": wqkvT,
            "bqkv": bqg,
            "relq": np.ascontiguousarray(relq_rev),
            "relk": np.ascontiguousarray(relk_rev),
            "relvT": relvT.astype(BF),
            "w3T": w3T,
            "wx1T": wx1T,
            "bx": b_x[VP * g:VP * (g + 1), None].astype(np.float32),
            "wruT": wruT,
            "wrbT": wrbT,
            "br": np.ascontiguousarray(b_r.reshape(4, 128).T).astype(np.float32),
            "wpT": wpT,
            "bp": b_p[VP * g:VP * (g + 1), None].astype(np.float32),
            "bsum": (bo[g, :, 0] + bo[g, :, 1])[:, None].astype(np.float32),
            "identb": np.eye(128, dtype=BF),
            "identf": np.eye(128, dtype=np.float32),
        })
    return in_maps


def run(inputs, trace=False):
    global _CACHED_NC
    _install_profile_hook()
    from concourse.bass_utils import run_bass_kernel_spmd
    if _CACHED_NC is None:
        _CACHED_NC = _build()
    in_maps = _host_prep(**inputs)
    res = run_bass_kernel_spmd(_CACHED_NC, in_maps, list(range(G)), trace=trace)
    out = np.concatenate(
        [res.results[g]["out"].reshape(NB, VP, 28, 28) for g in range(G)], axis=1)
    return out.astype(np.float32), res


def kernel(**inputs):
    out, _ = run(inputs, trace=False)
    return out


# revision 14
# speedup vs baseline: 1.3637x; 1.2654x over previous
"""nn_LESA Trainium2 kernel: 8-core SPMD Bass/Tile implementation.

Sharding: attention-group parallel (core g owns head-group g for all 4
batches: qkv projection slice, relative-position attention, grouped-conv
group, and the 64-channel output slice). Two small collectives knit the
full-channel 1x1-conv chain together: an AllGather of the grouped-conv
output and a per-batch AllReduce of the W_r partial products.

The relative-position einsums (qr/kr/sve) are computed as dense GEMMs
against `relative` in (i, t=i-j+HW-1) coordinates, then mapped back to
(i, j) with diagonal SBUF->SBUF DMA access patterns (per-partition
shifted windows); kr additionally needs PE transposes, as does attn for
the sv/sve contractions.
"""
import os
import sys
import types

import numpy as np
import ml_dtypes

import concourse.bass as bass
from concourse import mybir
from concourse.tile import TileContext

G = 8
NB = 4          # batches
QK = 32
VP = 64
HW = 784
T = 112         # pixel tile
NT = 7          # HW // T
WW = 896        # padded shear window width (8*T)
BR = 512
EPS = 1e-5
SHIFT = 20.0    # exp(x - SHIFT); cancels in normalization

f32 = mybir.dt.float32
bf16 = mybir.dt.bfloat16
BF = ml_dtypes.bfloat16

_DEBUG = bool(int(os.environ.get("BASSK_DEBUG", "0")))


def _split_multiwaits(nc):
    """walrus in this env allows 1 sync-wait per instruction; hoist extras
    onto same-engine NOPs placed just before the waiting instruction."""
    for f in nc.m.functions:
        for b in f.blocks:
            out = []
            for inst in b.instructions:
                si = inst.sync_info
                if si is not None and len(si.on_wait) > 1:
                    waits = list(si.on_wait)
                    for k, w in enumerate(waits[:-1]):
                        nop = mybir.InstNoOp(name=f"{inst.name}_w{k}", ins=[], outs=[])
                        nop.engine = inst.engine
                        nop.sync_info = mybir.SyncInfo(on_wait=[w], on_update=[])
                        out.append(nop)
                    inst.sync_info = mybir.SyncInfo(
                        on_wait=waits[-1:], on_update=list(si.on_update))
                out.append(inst)
            b.instructions = out


def _install_profile_hook():
    """antenv.axon_hooks is absent in this image; synthesize it so
    run_bass_kernel_spmd(trace=True) can reach the ctypes NTFF hook."""
    if "antenv.axon_hooks" in sys.modules:
        return
    try:
        from trn_agent_boot.trn_boot import _ntff_profile_via_ctypes
        hook = _ntff_profile_via_ctypes("/opt/axon/libaxon_pjrt.so")
    except Exception:
        hook = None
    mod = types.ModuleType("antenv.axon_hooks")
    mod.get_axon_ntff_profile_hook = lambda: hook
    mod.set_axon_ntff_profile_hook = lambda h: None
    sys.modules["antenv.axon_hooks"] = mod


def _diag(t_ap, extra_off, pstep, count, fstep, fcount):
    return bass.AP(t_ap.tensor, t_ap.offset + extra_off,
                   [[pstep, count], [fstep, fcount]])


def _build():
    nc = bass.Bass(num_devices=G)
    dt_in = {}

    def inp(name, shape, dt):
        dt_in[name] = nc.dram_tensor(name, list(shape), dt, kind="ExternalInput")
        return dt_in[name]

    xf = inp("xf", (NB, BR, HW), f32)
    xg = inp("xg", (NB, VP, HW), bf16)
    wqkvT = inp("wqkvT", (4, 128, 128), f32)
    bqkv = inp("bqkv", (128, 1), f32)
    relq = inp("relq", (QK, 2 * HW - 1), f32)
    relk = inp("relk", (QK, 2 * HW - 1), f32)
    relvT = inp("relvT", (2 * HW, VP), bf16)      # row 2*HW-1 zero-padded
    w3T = inp("w3T", (9, VP, VP), bf16)
    wx1T = inp("wx1T", (4, 128, VP), bf16)
    bx = inp("bx", (VP, 1), f32)
    wruT = inp("wruT", (VP, BR), bf16)
    wrbT = inp("wrbT", (VP, BR), bf16)
    br = inp("br", (128, 4), f32)
    wpT = inp("wpT", (4, 128, VP), bf16)
    bp = inp("bp", (VP, 1), f32)
    bsum = inp("bsum", (VP, 1), f32)
    identb = inp("identb", (128, 128), bf16)
    identf = inp("identf", (128, 128), f32)

    out_d = nc.dram_tensor("out", [NB, VP, HW], f32, kind="ExternalOutput")
    dbg = {}
    if _DEBUG:
        dbg["qkv0"] = nc.dram_tensor("dbg_qkv0", [128, HW], f32, kind="ExternalOutput")
        dbg["sim0"] = nc.dram_tensor("dbg_sim0", [T, HW], f32, kind="ExternalOutput")
        dbg["attn0"] = nc.dram_tensor("dbg_attn0", [T, WW], f32, kind="ExternalOutput")
        dbg["ash0"] = nc.dram_tensor("dbg_ash0", [T, WW], f32, kind="ExternalOutput")
        dbg["bin0"] = nc.dram_tensor("dbg_bin0", [VP, HW], f32, kind="ExternalOutput")
        dbg["u30"] = nc.dram_tensor("dbg_u30", [VP, HW], f32, kind="ExternalOutput")
        dbg["un0"] = nc.dram_tensor("dbg_un0", [VP, HW], f32, kind="ExternalOutput")
        dbg["gate0"] = nc.dram_tensor("dbg_gate0", [VP, HW], f32, kind="ExternalOutput")
        dbg["sve0"] = nc.dram_tensor("dbg_sve0", [VP, HW], f32, kind="ExternalOutput")
        dbg["sv0"] = nc.dram_tensor("dbg_sv0", [VP, HW], f32, kind="ExternalOutput")

    AF = mybir.ActivationFunctionType
    OP = mybir.AluOpType

    from contextlib import ExitStack
    with TileContext(nc) as tc, ExitStack() as stk:
        cst = stk.enter_context(tc.tile_pool(name="cst", bufs=1))
        wrk = stk.enter_context(tc.tile_pool(name="wrk", bufs=2))
        drp = stk.enter_context(tc.tile_pool(name="drp", bufs=1, space="DRAM"))
        # PSUM: sim (3 banks) + small pool (5 banks) = 8
        ps_sim = stk.enter_context(tc.tile_pool(name="ps_sim", bufs=3, space="PSUM"))
        ps_sm = stk.enter_context(tc.tile_pool(name="ps_sm", bufs=5, space="PSUM"))

        def smt(p, q, dt, name):
            return ps_sm.tile([p, q], dt, tag="sm", name=name)

        # ---------------- constants into SBUF ----------------
        xpad = cst.tile([VP, NB * 900], bf16, name="xpad")
        nc.gpsimd.memset(xpad[:], 0.0)
        for b in range(NB):
            dst = bass.AP(xpad[:].tensor, xpad[:].offset + b * 900 + 31,
                          [[NB * 900, VP], [30, 28], [1, 28]])
            nc.sync.dma_start(dst, xg[b])
        wqkvT_sb = cst.tile([128, 512], f32, name="wqkvT_sb")
        nc.sync.dma_start(
            wqkvT_sb[:].rearrange("p (k f) -> p k f", k=4),
            bass.AP(wqkvT, 0, [[128, 128], [128 * 128, 4], [1, 128]]))
        bqkv_sb = cst.tile([128, 1], f32, name="bqkv_sb")
        nc.sync.dma_start(bqkv_sb[:], bqkv[:])
        relq_sb = cst.tile([QK, 2 * HW - 1], f32, name="relq_sb")
        nc.sync.dma_start(relq_sb[:], relq[:])
        relk_sb = cst.tile([QK, 2 * HW - 1], f32, name="relk_sb")
        nc.sync.dma_start(relk_sb[:], relk[:])
        relvT_sb = cst.tile([T, 14 * VP], bf16, name="relvT_sb")
        nc.sync.dma_start(
            relvT_sb[:].rearrange("p (k f) -> p k f", k=14),
            bass.AP(relvT, 0, [[VP, T], [T * VP, 14], [1, VP]]))
        w3T_sb = cst.tile([VP, 9 * VP], bf16, name="w3T_sb")
        nc.sync.dma_start(
            w3T_sb[:].rearrange("p (k f) -> p k f", k=9),
            bass.AP(w3T, 0, [[VP, VP], [VP * VP, 9], [1, VP]]))
        wx1T_sb = cst.tile([128, 4 * VP], bf16, name="wx1T_sb")
        nc.sync.dma_start(
            wx1T_sb[:].rearrange("p (k f) -> p k f", k=4),
            bass.AP(wx1T, 0, [[VP, 128], [128 * VP, 4], [1, VP]]))
        wruT_sb = cst.tile([VP, BR], bf16, name="wruT_sb")
        nc.sync.dma_start(wruT_sb[:], wruT[:])
        wrbT_sb = cst.tile([VP, BR], bf16, name="wrbT_sb")
        nc.sync.dma_start(wrbT_sb[:], wrbT[:])
        wpT_sb = cst.tile([128, 4 * VP], bf16, name="wpT_sb")
        nc.sync.dma_start(
            wpT_sb[:].rearrange("p (k f) -> p k f", k=4),
            bass.AP(wpT, 0, [[VP, 128], [128 * VP, 4], [1, VP]]))
        bx_sb = cst.tile([VP, 1], f32, name="bx_sb")
        nc.sync.dma_start(bx_sb[:], bx[:])
        br_sb = cst.tile([128, 4], f32, name="br_sb")
        nc.sync.dma_start(br_sb[:], br[:])
        bp_sb = cst.tile([VP, 1], f32, name="bp_sb")
        nc.sync.dma_start(bp_sb[:], bp[:])
        bsum_sb = cst.tile([VP, 1], f32, name="bsum_sb")
        nc.sync.dma_start(bsum_sb[:], bsum[:])
        idb_sb = cst.tile([128, 128], bf16, name="idb_sb")
        nc.sync.dma_start(idb_sb[:], identb[:])
        idf_sb = cst.tile([128, 128], f32, name="idf_sb")
        nc.sync.dma_start(idf_sb[:], identf[:])
        zero_sb = cst.tile([128, 1], f32, name="zero_sb")
        nc.gpsimd.memset(zero_sb[:], 0.0)
        nshift_sb = cst.tile([128, 1], f32, name="nshift_sb")
        nc.gpsimd.memset(nshift_sb[:], -SHIFT)

        # persistent per-batch tiles
        q_sb = [cst.tile([QK, HW], f32, name=f"q{b}") for b in range(NB)]
        k_sb = [cst.tile([QK, HW], f32, name=f"k{b}") for b in range(NB)]
        v_sb = [cst.tile([VP, HW], bf16, name=f"v{b}") for b in range(NB)]
        vT_b = [cst.tile([T, NT * VP], bf16, name=f"vT{b}") for b in range(NB)]
        unary_b = [cst.tile([VP, HW], f32, name=f"un{b}") for b in range(NB)]
        ru_b = [cst.tile([VP, HW], bf16, name=f"ru{b}") for b in range(NB)]
        bin_b = [cst.tile([VP, HW], f32, name=f"bin{b}") for b in range(NB)]
        rb_b = [cst.tile([VP, HW], bf16, name=f"rb{b}") for b in range(NB)]

        # collective buffers (DRAM pool tiles so Tile tracks deps)
        cc1_in = drp.tile([NB * VP, HW], bf16, name="cc1_in")
        cc1_out = drp.tile([G * NB * VP, HW], bf16, name="cc1_out",
                           addr_space="Shared")
        cc2_in = [drp.tile([BR, HW], bf16, name=f"cc2i{b}") for b in range(NB)]
        cc2_out = [drp.tile([BR, HW], bf16, name=f"cc2o{b}",
                            addr_space="Shared") for b in range(NB)]

        NCH = (448, 336)  # HW split, bank-aligned psum chunks

        # ---------------- qkv projection + conv3x3 (all batches) ------------
        for b in range(NB):
            p0 = smt(128, 392, f32, f"qv0_{b}")
            p1 = smt(128, 392, f32, f"qv1_{b}")
            for kt in range(4):
                rhs = wrk.tile([128, HW], f32, name="xft", tag="xft", bufs=2)
                nc.sync.dma_start(
                    rhs[:], bass.AP(xf, b * BR * HW + kt * 128 * HW,
                                    [[HW, 128], [1, HW]]))
                lhsT = wqkvT_sb[:, kt * 128:(kt + 1) * 128]
                nc.tensor.matmul(p0[:], lhsT, rhs[:, 0:392],
                                 start=(kt == 0), stop=(kt == 3))
                nc.tensor.matmul(p1[:], lhsT, rhs[:, 392:HW],
                                 start=(kt == 0), stop=(kt == 3))
            qkv_f = wrk.tile([128, HW], f32, name="qkv_f", tag="qkv_f")
            nc.scalar.activation(qkv_f[:, 0:392], p0[:], AF.Identity,
                                 bias=bqkv_sb[:, 0:1])
            nc.scalar.activation(qkv_f[:, 392:HW], p1[:], AF.Identity,
                                 bias=bqkv_sb[:, 0:1])
            # partition-rebase q/k/v to base 0 (matmul needs equal bases)
            nc.sync.dma_start(q_sb[b][:], qkv_f[0:QK, :])
            nc.sync.dma_start(k_sb[b][:], qkv_f[QK:2 * QK, :])
            nc.gpsimd.dma_start(v_sb[b][:], qkv_f[2 * QK:128, :])
            # grouped 3x3 conv (unary branch pre-1x1)
            c0 = smt(VP, 392, f32, f"cv0_{b}")
            c1 = smt(VP, 392, f32, f"cv1_{b}")
            for k in range(9):
                dy, dx = divmod(k, 3)
                lhsT = w3T_sb[:, k * VP:(k + 1) * VP]
                for h, cp in ((0, c0), (1, c1)):
                    rhs = bass.AP(xpad[:].tensor,
                                  xpad[:].offset + b * 900 + dy * 30 + dx + h * 420,
                                  [[NB * 900, VP], [30, 14], [1, 28]])
                    nc.tensor.matmul(cp[:], lhsT, rhs,
                                     start=(k == 0), stop=(k == 8))
            u3 = wrk.tile([VP, HW], bf16, name="u3", tag="u3")
            nc.scalar.activation(u3[:, 0:392], c0[:], AF.Copy)
            nc.scalar.activation(u3[:, 392:HW], c1[:], AF.Copy)
            if _DEBUG and b == 0:
                nc.gpsimd.dma_start(dbg["u30"][:], u3[:])
                pass
            nc.sync.dma_start(cc1_in[b * VP:(b + 1) * VP, :], u3[:])
            # v reversed copy then plain transposes -> vT (descending j chunks)
            vrev = wrk.tile([VP, HW], bf16, name="vrev", tag="vrev")
            rev_out = bass.AP(vrev[:].tensor, vrev[:].offset + HW - 1,
                              [[HW, VP], [-1, HW]])
            nc.scalar.activation(rev_out, v_sb[b][:], AF.Copy)
            for j in range(NT):
                pt = smt(T, VP, bf16, f"vtp_{b}_{j}")
                nc.tensor.transpose(pt[:], vrev[:, j * T:(j + 1) * T],
                                    idb_sb[0:VP, 0:VP])
                nc.vector.tensor_copy(vT_b[b][:, j * VP:(j + 1) * VP], pt[:])

        nc.gpsimd.collective_compute(
            "AllGather", OP.bypass, replica_groups=[list(range(G))],
            ins=[cc1_in[:]], outs=[cc1_out[:]])

        # ---------------- attention + phase-2 part 1, per batch ----------
        for b in range(NB):
            # kr precompute: a_t_rev GEMM windows + shear -> A[j, i] rows
            ash_f = []
            for J in range(NT):
                w0 = 672 - J * T
                a0 = smt(T, 448, f32, f"at0_{b}_{J}")
                a1 = smt(T, 447, f32, f"at1_{b}_{J}")
                lhsT = k_sb[b][:, J * T:(J + 1) * T]
                nc.tensor.matmul(a0[:], lhsT, relk_sb[:, w0:w0 + 448],
                                 start=True, stop=True)
                nc.tensor.matmul(a1[:], lhsT, relk_sb[:, w0 + 448:w0 + 895],
                                 start=True, stop=True)
                araw = wrk.tile([T, WW], bf16, name="araw", tag="araw", bufs=3)
                nc.scalar.activation(araw[:, 0:448], a0[:], AF.Copy)
                nc.scalar.activation(araw[:, 448:895], a1[:], AF.Copy)
                af = wrk.tile([T, HW], bf16, name=f"ashf{J}", tag=f"ashf{J}",
                              bufs=2)
                nc.sync.dma_start(af[:], _diag(araw[:], 111, WW - 1, T, 1, HW))
                ash_f.append(af)

            def emit_stage_a(I):
                w0 = 672 - I * T
                q0 = smt(T, 448, f32, f"qr0_{b}_{I}")
                q1 = smt(T, 447, f32, f"qr1_{b}_{I}")
                lq = q_sb[b][:, I * T:(I + 1) * T]
                nc.tensor.matmul(q0[:], lq, relq_sb[:, w0:w0 + 448],
                                 start=True, stop=True)
                nc.tensor.matmul(q1[:], lq, relq_sb[:, w0 + 448:w0 + 895],
                                 start=True, stop=True)
                qraw = wrk.tile([T, WW], bf16, name="qraw", tag="qraw")
                nc.scalar.activation(qraw[:, 0:448], q0[:], AF.Copy)
                nc.scalar.activation(qraw[:, 448:895], q1[:], AF.Copy)
                qsh = wrk.tile([T, HW], bf16, name="qsh", tag="qsh")
                nc.sync.dma_start(qsh[:], _diag(qraw[:], 111, WW - 1, T, 1, HW))
                s0 = ps_sim.tile([T, 392], f32, tag="sim", name=f"s0_{b}_{I}")
                s1 = ps_sim.tile([T, 392], f32, tag="sim", name=f"s1_{b}_{I}")
                nc.tensor.matmul(s0[:], lq, k_sb[b][:, 0:392],
                                 start=True, stop=True, skip_group_check=True)
                nc.tensor.matmul(s1[:], lq, k_sb[b][:, 392:HW],
                                 start=True, stop=True, skip_group_check=True)
                kr_ps = smt(T, WW, bf16, f"krp_{b}_{I}")
                for J in range(NT):
                    nc.tensor.matmul(
                        kr_ps[:, J * T:(J + 1) * T],
                        ash_f[J][:, I * T:(I + 1) * T],
                        idb_sb[0:T, 0:T],
                        is_transpose=True, start=True, stop=True,
                        skip_group_check=True)
                sim = wrk.tile([T, HW], f32, name="sim", tag="sim")
                nc.vector.tensor_tensor(sim[:, 0:392], s0[:], qsh[:, 0:392], OP.add)
                nc.vector.tensor_tensor(sim[:, 392:HW], s1[:], qsh[:, 392:HW], OP.add)
                nc.vector.tensor_tensor(sim[:, 0:392], sim[:, 0:392],
                                        kr_ps[:, 0:392], OP.add)
                nc.vector.tensor_tensor(sim[:, 392:HW], sim[:, 392:HW],
                                        kr_ps[:, 392:HW], OP.add)
                if _DEBUG and b == 0 and I == 0:
                    nc.sync.dma_start(dbg["sim0"][:], sim[:])
                # exp with row-sum accumulation (normalization deferred)
                attn = wrk.tile([T, T + WW], bf16, name="attn", tag="attn", bufs=2)
                nc.any.memset(attn[:, 0:T], 0.0)
                nc.any.memset(attn[:, T + HW:T + WW], 0.0)
                nrm = wrk.tile([T, 1], f32, name="nrm", tag="nrm")
                rev = bass.AP(attn[:].tensor, attn[:].offset + T + HW - 1,
                              [[T + WW, T], [-1, HW]])
                nc.scalar.activation(rev, sim[:], AF.Exp,
                                     bias=nshift_sb[0:T, 0:1],
                                     accum_out=nrm[:])
                inv = wrk.tile([T, 1], f32, name="inv", tag="nrm")
                nc.vector.reciprocal(inv[:], nrm[:])
                return attn, inv

            def emit_stage_c(I, attn, inv):
                attnT = wrk.tile([T, HW], bf16, name="attnT", tag="attnT", bufs=2)
                for J in range(NT):
                    pt = smt(T, T, bf16, f"atT_{b}_{I}_{J}")
                    nc.tensor.transpose(pt[:], attn[:, T + J * T:T + (J + 1) * T],
                                        idb_sb[0:T, 0:T])
                    nc.vector.tensor_copy(attnT[:, J * T:(J + 1) * T], pt[:])
                ash = wrk.tile([T, WW], bf16, name="ash", tag="ash")
                nc.sync.dma_start(
                    ash[:], _diag(attn[:], T, T + WW - 1, T, 1, WW))
                ashT = wrk.tile([T, WW], bf16, name="ashT", tag="ashT")
                for uc in range(8):
                    pt = smt(T, T, bf16, f"ashT_{b}_{I}_{uc}")
                    nc.tensor.transpose(pt[:], ash[:, uc * T:(uc + 1) * T],
                                        idb_sb[0:T, 0:T])
                    nc.vector.tensor_copy(ashT[:, uc * T:(uc + 1) * T], pt[:])
                bp_ps = smt(T, VP, f32, f"bin_{b}_{I}")
                for uc in range(8):
                    s = I + uc
                    nc.tensor.matmul(bp_ps[:], ashT[:, uc * T:(uc + 1) * T],
                                     relvT_sb[:, s * VP:(s + 1) * VP],
                                     start=(uc == 0), stop=False,
                                     skip_group_check=True)
                for J in range(NT):
                    nc.tensor.matmul(bp_ps[:], attnT[:, J * T:(J + 1) * T],
                                     vT_b[b][:, J * VP:(J + 1) * VP],
                                     start=False, stop=(J == NT - 1),
                                     skip_group_check=True)
                # normalize per-row (i on partitions), transpose to [c, i]
                bpn = wrk.tile([T, VP], bf16, name="bpn", tag="bpn", bufs=2)
                nc.vector.tensor_scalar(bpn[:], bp_ps[:], inv[:], None, OP.mult)
                bpt = smt(VP, T, bf16, f"bpt_{b}_{I}")
                nc.tensor.transpose(bpt[:], bpn[:], idb_sb[0:T, 0:T])
                nc.scalar.activation(bin_b[b][:, I * T:(I + 1) * T], bpt[:],
                                     AF.Identity, bias=bsum_sb[:, 0:1])
                nc.scalar.activation(rb_b[b][:, I * T:(I + 1) * T], bpt[:],
                                     AF.Relu, bias=bsum_sb[:, 0:1])

            prev = None
            for I in range(NT):
                cur = emit_stage_a(I)
                if prev is not None:
                    emit_stage_c(I - 1, *prev)
                prev = cur
            emit_stage_c(NT - 1, *prev)

            if _DEBUG and b == 0:
                nc.sync.dma_start(dbg["bin0"][:], bin_b[0][:])

            # ---- phase 2 part 1: unary 1x1, W_r partials, AllReduce issue ----
            u0 = smt(VP, 392, f32, f"u0_{b}")
            u1p = smt(VP, 392, f32, f"u1_{b}")
            for kt in range(4):
                u3f = wrk.tile([128, HW], bf16, name="u3f", tag="u3f", bufs=3)
                for h in range(2):
                    r0_ = ((2 * kt + h) * NB + b) * VP
                    nc.sync.dma_start(u3f[h * VP:(h + 1) * VP, :],
                                      cc1_out[r0_:r0_ + VP, :])
                lhsT = wx1T_sb[:, kt * VP:(kt + 1) * VP]
                nc.tensor.matmul(u0[:], lhsT, u3f[:, 0:392],
                                 start=(kt == 0), stop=(kt == 3))
                nc.tensor.matmul(u1p[:], lhsT, u3f[:, 392:HW],
                                 start=(kt == 0), stop=(kt == 3))
            nc.scalar.activation(unary_b[b][:, 0:392], u0[:], AF.Identity,
                                 bias=bx_sb[:, 0:1])
            nc.scalar.activation(unary_b[b][:, 392:HW], u1p[:], AF.Identity,
                                 bias=bx_sb[:, 0:1])
            nc.scalar.activation(ru_b[b][:], unary_b[b][:], AF.Relu,
                                 bias=zero_sb[0:VP, 0:1])
            if _DEBUG and b == 0:
                nc.sync.dma_start(dbg["un0"][:], unary_b[0][:])
            for mt in range(4):
                r0 = smt(128, 392, f32, f"r0_{b}_{mt}")
                r1 = smt(128, 392, f32, f"r1_{b}_{mt}")
                lu = wruT_sb[:, mt * 128:(mt + 1) * 128]
                lb = wrbT_sb[:, mt * 128:(mt + 1) * 128]
                nc.tensor.matmul(r0[:], lu, ru_b[b][:, 0:392], start=True, stop=False)
                nc.tensor.matmul(r0[:], lb, rb_b[b][:, 0:392], start=False, stop=True)
                nc.tensor.matmul(r1[:], lu, ru_b[b][:, 392:HW], start=True, stop=False)
                nc.tensor.matmul(r1[:], lb, rb_b[b][:, 392:HW], start=False, stop=True)
                rp = wrk.tile([128, HW], bf16, name="rp", tag="rp")
                nc.scalar.activation(rp[:, 0:392], r0[:], AF.Copy)
                nc.scalar.activation(rp[:, 392:HW], r1[:], AF.Copy)
                nc.sync.dma_start(cc2_in[b][mt * 128:(mt + 1) * 128, :], rp[:])
            nc.gpsimd.collective_compute(
                "AllReduce", OP.add, replica_groups=[list(range(G))],
                ins=[cc2_in[b][:]], outs=[cc2_out[b][:]])

        # ---------------- phase 2 part 2: gate + output ----------------
        for b in range(NB):
            g0 = smt(VP, 392, f32, f"g0_{b}")
            g1 = smt(VP, 392, f32, f"g1_{b}")
            for kt in range(4):
                rr = wrk.tile([128, HW], bf16, name="rr", tag="rr", bufs=3)
                nc.sync.dma_start(rr[:], cc2_out[b][kt * 128:(kt + 1) * 128, :])
                rrl = wrk.tile([128, HW], bf16, name="rrl", tag="rrl", bufs=3)
                nc.scalar.activation(rrl[:], rr[:], AF.Relu,
                                     bias=br_sb[:, kt:kt + 1])
                lhsT = wpT_sb[:, kt * VP:(kt + 1) * VP]
                nc.tensor.matmul(g0[:], lhsT, rrl[:, 0:392],
                                 start=(kt == 0), stop=(kt == 3))
                nc.tensor.matmul(g1[:], lhsT, rrl[:, 392:HW],
                                 start=(kt == 0), stop=(kt == 3))
            gate = wrk.tile([VP, HW], f32, name="gate", tag="gate")
            nc.scalar.activation(gate[:, 0:392], g0[:], AF.Sigmoid,
                                 bias=bp_sb[:, 0:1])
            nc.scalar.activation(gate[:, 392:HW], g1[:], AF.Sigmoid,
                                 bias=bp_sb[:, 0:1])
            if _DEBUG and b == 0:
                nc.sync.dma_start(dbg["gate0"][:], gate[:])
            nc.vector.tensor_tensor(gate[:], gate[:], bin_b[b][:], OP.mult)
            nc.vector.tensor_tensor(gate[:], gate[:], unary_b[b][:], OP.add)
            nc.sync.dma_start(out_d[b], gate[:])

    _split_multiwaits(nc)
    return nc


_CACHED_NC = None


def _host_prep(x, W_qkv, g_qkv, b_qkv, relative, g_sim, b_sim, g_out, b_out,
               W_x3, W_x1, g_x, b_x, W_r, g_r, b_r, W_p, g_p, b_p):
    sc = 1.0 / np.sqrt(1.0 + EPS)
    s_qkv = (np.float32(g_qkv) * sc).astype(np.float32)
    bq = np.float32(b_qkv)
    s_sim = (np.float32(g_sim) * sc).astype(np.float32)
    s_out = (np.float32(g_out) * sc).astype(np.float32).reshape(G, VP, 2)
    bo = np.float32(b_out).reshape(G, VP, 2)
    s_x = (np.float32(g_x) * sc); s_r = (np.float32(g_r) * sc)
    s_p = (np.float32(g_p) * sc)
    Wq = np.float32(W_qkv)
    xf = np.ascontiguousarray(np.float32(x).reshape(NB, BR, HW))
    rel = np.float32(relative)
    W3 = np.float32(W_x3).reshape(G, VP, VP, 3, 3)
    Wx1 = np.float32(W_x1)[:, :, 0, 0]
    Wr = np.float32(W_r)[:, :, 0, 0]
    Wp = np.float32(W_p)[:, :, 0, 0]
    b_x = np.float32(b_x); b_r = np.float32(b_r); b_p = np.float32(b_p)

    in_maps = []
    for g in range(G):
        s0, s1, s2 = s_sim[g], s_sim[G + g], s_sim[2 * G + g]
        a = np.sign(s0) * np.sqrt(max(abs(s0), 1e-30))
        be = np.sqrt(max(abs(s0), 1e-30))
        rows = np.empty(128, np.float32)
        rows[0:QK] = a
        rows[QK:2 * QK] = be
        rows[2 * QK:] = s_out[g, :, 0]
        base = 128 * g
        Wg = Wq[base:base + 128] * (rows * s_qkv[base:base + 128])[:, None]
        bqg = (bq[base:base + 128] * rows).astype(np.float32)[:, None]
        wqkvT = np.ascontiguousarray(
            Wg.T.reshape(4, 128, 128)).astype(np.float32)   # [kt, cc, o]
        relq_rev = (rel[0:QK, ::-1] * (s1 / a)).astype(np.float32)
        relk_rev = (rel[QK:2 * QK, ::-1] * (s2 / be)).astype(np.float32)
        relvT = np.zeros((2 * HW, VP), np.float32)
        relvT[0:2 * HW - 1] = rel[2 * QK:].T * s_out[g, :, 1][None, :]
        w3T = np.ascontiguousarray(
            W3[g].transpose(2, 3, 1, 0).reshape(9, VP, VP)).astype(BF)
        wx1T = np.ascontiguousarray(
            (Wx1[VP * g:VP * (g + 1)] * s_x[VP * g:VP * (g + 1), None]).T
            .reshape(4, 128, VP)).astype(BF)
        wruT = np.ascontiguousarray(
            (Wr[:, VP * g:VP * (g + 1)] * s_r[:, None]).T).astype(BF)
        wrbT = np.ascontiguousarray(
            (Wr[:, BR + VP * g:BR + VP * (g + 1)] * s_r[:, None]).T).astype(BF)
        wpT = np.ascontiguousarray(
            (Wp[VP * g:VP * (g + 1)] * s_p[VP * g:VP * (g + 1), None]).T
            .reshape(4, 128, VP)).astype(BF)
        in_maps.append({
            "xf": xf,
            "xg": np.ascontiguousarray(
                np.float32(x).reshape(NB, BR, HW)[:, VP * g:VP * (g + 1)]).astype(BF),
            "wqkvT": wqkvT,
            "bqkv": bqg,
            "relq": np.ascontiguousarray(relq_rev),
            "relk": np.ascontiguousarray(relk_rev),
            "relvT": relvT.astype(BF),
            "w3T": w3T,
            "wx1T": wx1T,
            "bx": b_x[VP * g:VP * (g + 1), None].astype(np.float32),
            "wruT": wruT,
            "wrbT": wrbT,
            "br": np.ascontiguousarray(b_r.reshape(4, 128).T).astype(np.float32),
            "wpT": wpT,
            "bp": b_p[VP * g:VP * (g + 1), None].astype(np.float32),
            "bsum": (bo[g, :, 0] + bo[g, :, 1])[:, None].astype(np.float32),
            "identb": np.eye(128, dtype=BF),
            "identf": np.eye(128, dtype=np.float32),
        })
    return in_maps


def run(inputs, trace=False):
    global _CACHED_NC
    _install_profile_hook()
    from concourse.bass_utils import run_bass_kernel_spmd
    if _CACHED_NC is None:
        _CACHED_NC = _build()
    in_maps = _host_prep(**inputs)
    res = run_bass_kernel_spmd(_CACHED_NC, in_maps, list(range(G)), trace=trace)
    out = np.concatenate(
        [res.results[g]["out"].reshape(NB, VP, 28, 28) for g in range(G)], axis=1)
    return out.astype(np.float32), res


def kernel(**inputs):
    out, _ = run(inputs, trace=False)
    return out
